# revision 1
# baseline (speedup 1.0000x reference)
"""Bass/Tile kernel for a 4-layer decoder transformer + 32k-vocab LM head on 8 trn2 cores.

Sharding: data-parallel over batch in pairs (core c -> batch c//2), with the
lm_head vocab dim split across each pair (core c -> vocab half c%2). The tiny
transformer params are replicated; each core of a pair redundantly computes the
64-dim transformer for its sequence (cheaper than cross-core collectives), then
produces [1024, 16000] logits. Host reassembles [4, 1024, 32000].

Model: B=4, T=1024, C=64, H=4 heads x 16, L=4, FF=256, V=32000.

Loop order: 4-chunk groups x layers x phases, so same-table-set ScalarE
activations batch together (Sqrt for both LNs, Exp for softmax) -- ~16 ACT
table loads instead of 100+.

Per 128-token chunk i, per layer l:
  phase 1: LN (bn_stats, Sqrt+recip rstd) -> xn [128,65] with a ones column
    that folds every bias (ln_b via weight row 64, blm via Wlm row 64);
    PE transpose -> xnT [65,128]; per-head q/k at partition base 0 (row-tiled
    tile_position is broken on this terminal; col-tiling works), v in 32-wide
    head strips whose first column is the ones column of Wv (softmax denom).
  phase 2: transposed scores wT[s,t] per key-chunk j<=i; causal mask for the
    diagonal block accumulates into PSUM via an identity x (-1e9 mask) matmul;
    exp fused with PSUM evacuation on ScalarE (scale=1/8), bf16 out.
  phase 3: av accumulated over j with 4 heads col-tiled (head h at PSUM
    partitions 32h..32h+31; strict h-major groups -- start=True clears
    has_written bank-wide); den lands at row 32h; one static selector matmul
    (bsel) broadcasts den rows to whole strips, one reciprocal + one
    tensor_tensor multiply normalizes and casts to bf16; out-proj via
    zero-padded Wo kills the den/garbage rows; residual add.
  phase 4: LN2 + MLP (hT = relu(W1_pad.T @ xn2T) per 128-wide ff chunk with
    per-partition b1 bias on the Relu, y accumulated over ff chunks) +
    residual; at l=3 the chunk's lm_head is emitted inline so logits DMA
    overlaps the next group's compute: logits = xT_aug @ Wlm_aug in 500-col
    PSUM tiles, evacuation split VectorE/ScalarE, bf16 stores (halves the
    dominant HBM write; host upconverts to f32).
"""

import os

import numpy as np
import ml_dtypes

import concourse.bass as bass
import concourse.mybir as mybir
import concourse.tile as tile
from concourse import bacc
from concourse.bass import ts
from concourse.bass_utils import run_bass_kernel_spmd

# model dims
T = 1024
C = 64
H = 4
D = 16
L = 4
FF = 256
V = 32000
VH = V // 2          # per-core vocab half
NT = T // 128        # 8 token chunks
VC = 500             # vocab chunk (lm head matmul N)
NVC = VH // VC       # 32 vocab chunks per core
VG = 4               # vocab chunks per staged DMA (2000 f32 = 8KB/partition)
SCALE = 1.0 / (C ** 0.5)
NEG = -1.0e9

F32 = mybir.dt.float32
BF16 = mybir.dt.bfloat16
I32 = mybir.dt.int32

_CACHE = {}

# filled with the BassKernelResults of the last run (for test.py profiling)
LAST_RESULTS = None


def _build(apply_bo, apply_b2, parts='all'):
    nc = bacc.Bacc("TRN2", target_bir_lowering=False, debug=False)

    # ---- DRAM I/O ----
    idx_d = nc.dram_tensor("idx", [128, NT], I32, kind="ExternalInput").ap()
    temb_d = nc.dram_tensor("tok_emb", [V, C], F32, kind="ExternalInput").ap()
    pemb_d = nc.dram_tensor("pos_emb", [T, C], F32, kind="ExternalInput").ap()
    wq_d = nc.dram_tensor("wq", [L, 65, C], BF16, kind="ExternalInput").ap()
    wk_d = nc.dram_tensor("wk", [L, 65, C], BF16, kind="ExternalInput").ap()
    wv_d = nc.dram_tensor("wv", [L, 65, 128], BF16, kind="ExternalInput").ap()
    wo_d = nc.dram_tensor("wo", [L, 128, C], BF16, kind="ExternalInput").ap()
    w1_d = nc.dram_tensor("w1", [L, 65, FF], BF16, kind="ExternalInput").ap()
    w2_d = nc.dram_tensor("w2", [L, 2, 128, C], BF16, kind="ExternalInput").ap()
    b1_d = nc.dram_tensor("b1c", [128, L * 2], F32, kind="ExternalInput").ap()
    wlm_d = nc.dram_tensor("wlm", [65, VH], F32, kind="ExternalInput").ap()
    mask_d = nc.dram_tensor("maskt", [128, 128], BF16, kind="ExternalInput").ap()
    idb_d = nc.dram_tensor("identb", [128, 128], BF16, kind="ExternalInput").ap()
    idf_d = nc.dram_tensor("identf", [128, 128], F32, kind="ExternalInput").ap()
    bsel_d = nc.dram_tensor("bsel", [128, 128], F32, kind="ExternalInput").ap()
    bo_d = b2_d = None
    if apply_bo:
        bo_d = nc.dram_tensor("bo_bc", [L, C], F32, kind="ExternalInput").ap()
    if apply_b2:
        b2_d = nc.dram_tensor("b2_bc", [L, C], F32, kind="ExternalInput").ap()
    # logits stored bf16 (halves the dominant HBM write); host upconverts
    out_d = nc.dram_tensor("logits", [T, VH], BF16, kind="ExternalOutput").ap()

    with tile.TileContext(nc) as tc:
        with (
            tc.tile_pool(name="singles", bufs=1) as singles,
            tc.tile_pool(name="sb2", bufs=3) as sb2,
            tc.tile_pool(name="sb3", bufs=3) as sb3,
            tc.tile_pool(name="sb5", bufs=5) as sb5,
            tc.tile_pool(name="ln4", bufs=6) as ln4,
            tc.tile_pool(name="stage", bufs=4) as stagep,
            tc.tile_pool(name="wps", bufs=2, space="PSUM") as wps,
            tc.tile_pool(name="smallps", bufs=2, space="PSUM") as smallps,
            tc.tile_pool(name="lgps", bufs=2, space="PSUM") as lgps,
        ):
            # ---- resident SBUF tensors ----
            wq_sb = singles.tile([65, L, C], BF16, name="wq_sb")
            wk_sb = singles.tile([65, L, C], BF16, name="wk_sb")
            wv_sb = singles.tile([65, L, 128], BF16, name="wv_sb")
            wo_sb = singles.tile([128, L, C], BF16, name="wo_sb")
            w1_sb = singles.tile([65, L, FF], BF16, name="w1_sb")
            w2_sb = singles.tile([128, L, 2, C], BF16, name="w2_sb")
            b1_sb = singles.tile([128, L * 2], F32, name="b1_sb")
            wlm_sb = singles.tile([65, VH], F32, name="wlm_sb")
            mask_sb = singles.tile([128, 128], BF16, name="mask_sb")
            idb_sb = singles.tile([128, 128], BF16, name="idb_sb")
            idf_sb = singles.tile([128, 128], F32, name="idf_sb")
            idx_sb = singles.tile([128, NT], I32, name="idx_sb")
            eps_sb = singles.tile([128, 1], F32, name="eps_sb")
            bsel_sb = singles.tile([128, 128], F32, name="bsel_sb")
            x_sb = singles.tile([128, NT, 65], F32, name="x_sb")
            kcache = [singles.tile([16, H, NT, 128], BF16, name=f"kc{l}") for l in range(L)]
            vcache = [singles.tile([128, NT, 128], BF16, name=f"vc{l}") for l in range(L)]

            for l in range(L):
                nc.sync.dma_start(out=wq_sb[:, l, :], in_=wq_d[l])
                nc.sync.dma_start(out=wk_sb[:, l, :], in_=wk_d[l])
                nc.sync.dma_start(out=wv_sb[:, l, :], in_=wv_d[l])
                nc.sync.dma_start(out=wo_sb[:, l, :], in_=wo_d[l])
                nc.sync.dma_start(out=w1_sb[:, l, :], in_=w1_d[l])
                nc.sync.dma_start(out=w2_sb[:, l, 0, :], in_=w2_d[l, 0])
                nc.sync.dma_start(out=w2_sb[:, l, 1, :], in_=w2_d[l, 1])
            nc.sync.dma_start(out=b1_sb, in_=b1_d)
            nc.sync.dma_start(out=wlm_sb, in_=wlm_d)
            nc.sync.dma_start(out=mask_sb, in_=mask_d)
            nc.sync.dma_start(out=idb_sb, in_=idb_d)
            nc.sync.dma_start(out=idf_sb, in_=idf_d)
            nc.sync.dma_start(out=bsel_sb, in_=bsel_d)
            nc.sync.dma_start(out=idx_sb, in_=idx_d)
            nc.vector.memset(eps_sb, 1e-5)

            bo_sb = b2_sb = None
            if apply_bo:
                bo_sb = singles.tile([128, L, C], F32, name="bo_sb")
                for l in range(L):
                    row = bo_d[l: l + 1, :]
                    nc.gpsimd.dma_start(
                        out=bo_sb[:, l, :],
                        in_=bass.AP(tensor=row.tensor, offset=row.offset,
                                    ap=[[0, 128], [1, C]]),
                    )
            if apply_b2:
                b2_sb = singles.tile([128, L, C], F32, name="b2_sb")
                for l in range(L):
                    row = b2_d[l: l + 1, :]
                    nc.gpsimd.dma_start(
                        out=b2_sb[:, l, :],
                        in_=bass.AP(tensor=row.tensor, offset=row.offset,
                                    ap=[[0, 128], [1, C]]),
                    )

            # ---- embeddings: x = tok_emb[idx] + pos_emb, ones col ----
            for i in range(NT):
                nc.gpsimd.indirect_dma_start(
                    out=x_sb[:, i, 0:C],
                    out_offset=None,
                    in_=temb_d,
                    in_offset=bass.IndirectOffsetOnAxis(ap=idx_sb[:, i:i + 1], axis=0),
                )
                pos_t = sb3.tile([128, C], F32, name="pos_t")
                nc.sync.dma_start(out=pos_t, in_=pemb_d[ts(i, 128), :])
                nc.vector.tensor_add(out=x_sb[:, i, 0:C], in0=x_sb[:, i, 0:C], in1=pos_t)
            nc.vector.memset(x_sb[:, :, 64:65], 1.0)

            def layernorm(i, which, l):
                """x_sb[:,i,:64] -> normalized xn tile [128,65] (col 64 = 1)."""
                stats = ln4.tile([128, 6], F32, name="ln_stats")
                mv = ln4.tile([128, 2], F32, name="ln_mv")
                rstd = ln4.tile([128, 1], F32, name="ln_rstd")
                nc.vector.bn_stats(out=stats, in_=x_sb[:, i, 0:C])
                nc.vector.bn_aggr(out=mv, in_=stats)
                nc.scalar.activation(out=rstd, in_=mv[:, 1:2],
                                     func=mybir.ActivationFunctionType.Sqrt,
                                     bias=eps_sb)
                nc.vector.reciprocal(out=rstd, in_=rstd)
                xn = sb2.tile([128, 65], F32, name=f"xn{which}")
                nc.vector.tensor_scalar(
                    out=xn[:, 0:C], in0=x_sb[:, i, 0:C],
                    scalar1=mv[:, 0:1], scalar2=rstd,
                    op0=mybir.AluOpType.subtract, op1=mybir.AluOpType.mult,
                )
                nc.vector.memset(xn[:, 64:65], 1.0)
                return xn

            def transpose65(xn, name, dtype):
                """[128,65] -> bf16/f32 [65,128] in SBUF via PE transpose."""
                tp = smallps.tile([65, 128], F32, name="tp_ps", tag="smallps")
                nc.tensor.transpose(out=tp, in_=xn, identity=idf_sb)
                xt = sb2.tile([65, 128], dtype, name=name)
                nc.vector.tensor_copy(out=xt, in_=tp)
                return xt

            lvl = {'emb': 0, 'nolm': 6, 'all': 7}.get(parts, parts)
            do_lm = lvl >= 7
            G = 4
            qt_of, pt_of = {}, {}
            for grp_base in range(0, NT, G):
                grp = range(grp_base, min(grp_base + G, NT))
                for l in range(L if lvl >= 1 else 0):
                  # phase 1: LN1 (Sqrt) + transpose + q/k/v for the whole group
                  for i in grp:
                    xn = layernorm(i, 1, l)
                    xnt = transpose65(xn, "xnt", BF16)

                    if lvl < 2:
                        continue
                    q_ps = smallps.tile([16, 512], F32, name="q_ps", tag="smallps")
                    k_ps = smallps.tile([16, 512], F32, name="k_ps", tag="smallps")
                    for h in range(H):
                        nc.tensor.matmul(out=q_ps[:, ts(h, 128)],
                                         lhsT=wq_sb[:, l, 16 * h: 16 * h + D],
                                         rhs=xnt, start=True, stop=True)
                        nc.tensor.matmul(out=k_ps[:, ts(h, 128)],
                                         lhsT=wk_sb[:, l, 16 * h: 16 * h + D],
                                         rhs=xnt, start=True, stop=True)
                    qt = sb5.tile([16, 512], BF16, name="qt")
                    qt_of[i] = qt
                    nc.vector.tensor_copy(out=qt, in_=q_ps)
                    nc.scalar.copy(out=kcache[l][:, :, i, :], in_=k_ps)

                    v_ps = smallps.tile([128, 128], F32, name="v_ps", tag="smallps")
                    nc.tensor.matmul(out=v_ps, lhsT=xnt, rhs=wv_sb[:, l, :],
                                     start=True, stop=True)
                    nc.vector.tensor_copy(out=vcache[l][:, i, :], in_=v_ps)

                  # phase 2: scores + exp (one Exp table visit per group)
                  for i in grp:
                    if lvl < 3:
                        continue
                    qt = qt_of[i]
                    pt = sb5.tile([128, (i + 1) * 512], BF16, name="pt")
                    pt_of[i] = pt
                    for g in range(0, i + 1, 2):
                        js = [j for j in (g, g + 1) if j <= i]
                        w_ps = wps.tile([128, 512 * len(js)], F32, name="w_ps")
                        for j in js:
                            off = (j - g) * 512
                            for h in range(H):
                                o = w_ps[:, off + h * 128: off + (h + 1) * 128]
                                if j == i:
                                    nc.tensor.matmul(
                                        out=o, lhsT=idb_sb, rhs=mask_sb,
                                        start=True, stop=False)
                                nc.tensor.matmul(
                                    out=o,
                                    lhsT=kcache[l][:, h, j, :],
                                    rhs=qt[:, ts(h, 128)],
                                    start=(j != i), stop=True,
                                )
                        nc.scalar.activation(
                            out=pt[:, g * 512: g * 512 + 512 * len(js)],
                            in_=w_ps,
                            func=mybir.ActivationFunctionType.Exp, scale=SCALE)

                  # phase 3: av + normalize + out-proj + residual
                  for i in grp:
                    if lvl < 4:
                        continue
                    pt = pt_of[i]
                    at_ps = smallps.tile([128, 128], F32, name="at_ps", tag="smallps")
                    # h-major: start=True clears has_written for the whole
                    # bank, so each head's accumulation group must fully close
                    # before the next head's opens (finished values survive a
                    # bit-clear; open groups do not).
                    for h in range(H):
                        for j in range(i + 1):
                            nc.tensor.matmul(
                                out=at_ps[32 * h: 32 * h + 32, :],
                                lhsT=vcache[l][:, j, 32 * h: 32 * h + 32],
                                rhs=pt[:, (4 * j + h) * 128: (4 * j + h + 1) * 128],
                                start=(j == 0), stop=(j == i),
                                tile_position=(0, 32 * h),
                            )
                    at_un = sb2.tile([128, 128], F32, name="at_un")
                    nc.vector.tensor_copy(out=at_un, in_=at_ps)
                    # broadcast each head's den row (at partition 32h) to its whole
                    # 32-row strip with one selector matmul (bsel[k,p]=1 iff
                    # k==32*(p//32)), then one full-width reciprocal
                    den_ps = smallps.tile([128, 128], F32, name="den_ps",
                                          tag="smallps")
                    nc.tensor.matmul(out=den_ps, lhsT=bsel_sb, rhs=at_un,
                                     start=True, stop=True)
                    rec_sb = sb2.tile([128, 128], F32, name="rec_sb")
                    nc.vector.reciprocal(out=rec_sb, in_=den_ps)
                    at_n = sb2.tile([128, 128], BF16, name="at_n")
                    nc.vector.tensor_tensor(out=at_n, in0=at_un, in1=rec_sb,
                                            op=mybir.AluOpType.mult)

                    if lvl < 5:
                        continue
                    xo_ps = smallps.tile([128, C], F32, name="xo_ps", tag="smallps")
                    nc.tensor.matmul(out=xo_ps, lhsT=at_n, rhs=wo_sb[:, l, :],
                                     start=True, stop=True)
                    nc.vector.tensor_add(out=x_sb[:, i, 0:C], in0=x_sb[:, i, 0:C],
                                         in1=xo_ps)
                    if apply_bo:
                        nc.vector.tensor_add(out=x_sb[:, i, 0:C],
                                             in0=x_sb[:, i, 0:C], in1=bo_sb[:, l, :])

                  # phase 4: LN2 (Sqrt) + MLP for the whole group
                  for i in grp:
                    if lvl < 6:
                        continue
                    xn2 = layernorm(i, 2, l)
                    xn2t = transpose65(xn2, "xn2t", BF16)
                    y_ps = smallps.tile([128, C], F32, name="y_ps", tag="smallps")
                    h_ps = smallps.tile([128, 2, 128], F32, name="h_ps",
                                        tag="smallps")
                    for n in range(2):
                        nc.tensor.matmul(out=h_ps[:, n, :],
                                         lhsT=w1_sb[:, l, ts(n, 128)],
                                         rhs=xn2t, start=True, stop=True)
                    h_sb = sb2.tile([128, 2, 128], BF16, name="h_sb")
                    # both ff chunks share one relu; b1 is per-partition per
                    # chunk so it still applies per 128-wide slice
                    for n in range(2):
                        nc.scalar.activation(out=h_sb[:, n, :], in_=h_ps[:, n, :],
                                             func=mybir.ActivationFunctionType.Relu,
                                             bias=b1_sb[:, 2 * l + n: 2 * l + n + 1])
                    for n in range(2):
                        nc.tensor.matmul(out=y_ps, lhsT=h_sb[:, n, :],
                                         rhs=w2_sb[:, l, n, :],
                                         start=(n == 0), stop=(n == 1))
                    nc.vector.tensor_add(out=x_sb[:, i, 0:C], in0=x_sb[:, i, 0:C],
                                         in1=y_ps)
                    if apply_b2:
                        nc.vector.tensor_add(out=x_sb[:, i, 0:C],
                                             in0=x_sb[:, i, 0:C], in1=b2_sb[:, l, :])

                    # ---- lm head, emitted right after this chunk's last layer
                    if not (do_lm and l == L - 1):
                        continue
                    xt = transpose65(x_sb[:, i, :], "xt_lm", F32)
                    for kg in range(NVC // VG):
                        st = stagep.tile([128, VG * VC], BF16, name="lg_stage")
                        for u in range(VG):
                            k = kg * VG + u
                            lg_ps = lgps.tile([128, VC], F32, name="lg_ps")
                            nc.tensor.matmul(out=lg_ps, lhsT=xt,
                                             rhs=wlm_sb[:, k * VC:(k + 1) * VC],
                                             start=True, stop=True)
                            dst = st[:, u * VC:(u + 1) * VC]
                            if k % 8 < 3:
                                nc.vector.tensor_copy(out=dst, in_=lg_ps)
                            else:
                                nc.scalar.copy(out=dst, in_=lg_ps)
                        nc.sync.dma_start(
                            out=out_d[ts(i, 128), kg * VG * VC:(kg + 1) * VG * VC],
                            in_=st)
            if not do_lm:
                xdbg_d = nc.dram_tensor("xdbg", [128, NT * 65], F32,
                                        kind="ExternalOutput").ap()
                nc.sync.dma_start(out=xdbg_d,
                                  in_=x_sb.rearrange("p a b -> p (a b)"))
    nc.compile()
    return nc


def _prep_inputs(idx, tok_emb, pos_emb, Wq, Wk, Wv, Wo, bo, W1, b1, W2, b2,
                 ln1_g, ln1_b, ln2_g, ln2_b, Wlm, blm):
    """Host-side weight layout prep. Returns (common dict, per-core dicts, flags)."""
    f32 = np.float32
    bf16 = ml_dtypes.bfloat16
    Wq, Wk, Wv, Wo = f32(Wq), f32(Wk), f32(Wv), f32(Wo)
    W1, W2, Wlm = f32(W1), f32(W2), f32(Wlm)
    ln1_g, ln1_b, ln2_g, ln2_b = f32(ln1_g), f32(ln1_b), f32(ln2_g), f32(ln2_b)
    bo, b1, b2, blm = f32(bo), f32(b1), f32(b2), f32(blm)

    wq_np = np.zeros((L, 65, C), f32)
    wk_np = np.zeros((L, 65, C), f32)
    wv_np = np.zeros((L, 65, 128), f32)
    wo_np = np.zeros((L, 128, C), f32)
    w1_np = np.zeros((L, 65, FF), f32)
    for l in range(L):
        g1, b1n = ln1_g[l], ln1_b[l]
        g2, b2n = ln2_g[l], ln2_b[l]
        for h in range(H):
            wq_np[l, 0:C, 16 * h:16 * h + D] = g1[:, None] * Wq[l, h]
            wq_np[l, 64, 16 * h:16 * h + D] = b1n @ Wq[l, h]
            wk_np[l, 0:C, 16 * h:16 * h + D] = g1[:, None] * Wk[l, h]
            wk_np[l, 64, 16 * h:16 * h + D] = b1n @ Wk[l, h]
            wv_np[l, 0:C, 32 * h + 1:32 * h + 1 + D] = g1[:, None] * Wv[l, h]
            wv_np[l, 64, 32 * h + 1:32 * h + 1 + D] = b1n @ Wv[l, h]
            wv_np[l, 64, 32 * h] = 1.0
            wo_np[l, 32 * h + 1:32 * h + 1 + D, :] = Wo[l, 16 * h:16 * h + D, :]
        w1_np[l, 0:C, :] = g2[:, None] * W1[l]
        w1_np[l, 64, :] = b2n @ W1[l]
    w2_np = W2.reshape(L, 2, 128, C)
    b1c_np = np.ascontiguousarray(
        b1.reshape(L * 2, 128).T)  # [128, L*2]

    sidx = np.arange(128)
    mask_np = np.where(sidx[:, None] <= sidx[None, :], 0.0, NEG).astype(f32)
    ident_np = np.eye(128, dtype=f32)
    bsel_np = np.zeros((128, 128), f32)
    bsel_np[32 * (sidx // 32), sidx] = 1.0

    common = {
        "tok_emb": np.ascontiguousarray(tok_emb, f32),
        "pos_emb": np.ascontiguousarray(pos_emb, f32),
        "wq": wq_np.astype(bf16), "wk": wk_np.astype(bf16),
        "wv": wv_np.astype(bf16), "wo": wo_np.astype(bf16),
        "w1": w1_np.astype(bf16), "w2": w2_np.astype(bf16),
        "b1c": b1c_np,
        "maskt": mask_np.astype(bf16),
        "identb": ident_np.astype(bf16),
        "identf": ident_np,
        "bsel": bsel_np,
    }
    apply_bo = bool(np.any(bo != 0))
    apply_b2 = bool(np.any(b2 != 0))
    if apply_bo:
        common["bo_bc"] = np.ascontiguousarray(bo, f32)
    if apply_b2:
        common["b2_bc"] = np.ascontiguousarray(b2, f32)

    wlm_aug = np.concatenate([Wlm, blm[None, :]], axis=0)  # [65, V]
    idx_i = np.asarray(idx).astype(np.int32)

    per_core = []
    for c in range(8):
        b, half = c // 2, c % 2
        m = dict(common)
        m["idx"] = np.ascontiguousarray(idx_i[b].reshape(NT, 128).T)  # [128, NT]
        m["wlm"] = np.ascontiguousarray(wlm_aug[:, half * VH:(half + 1) * VH])
        per_core.append(m)
    return per_core, apply_bo, apply_b2


def kernel(**inputs):
    global LAST_RESULTS
    per_core, apply_bo, apply_b2 = _prep_inputs(**inputs)

    key = (apply_bo, apply_b2)
    if key not in _CACHE:
        _CACHE[key] = _build(apply_bo, apply_b2)
    nc = _CACHE[key]

    trace = os.environ.get("KERNEL_TRACE", "0") == "1"
    if trace:
        try:
            from antenv.axon_hooks import get_axon_ntff_profile_hook  # noqa: F401
        except ImportError:
            trace = False  # no NTFF path in this container
    res = run_bass_kernel_spmd(nc, per_core, core_ids=list(range(8)), trace=trace)
    LAST_RESULTS = res

    out = np.empty((4, T, V), np.float32)
    for c in range(8):
        b, half = c // 2, c % 2
        out[b, :, half * VH:(half + 1) * VH] = np.float32(res.results[c]["logits"])
    return out



# revision 8
# speedup vs baseline: 1.2686x; 1.2686x over previous
"""Bass/Tile kernel for a 4-layer decoder transformer + 32k-vocab LM head on 8 trn2 cores.

Sharding: data-parallel over batch in pairs (core c -> batch c//2), with the
lm_head vocab dim split across each pair (core c -> vocab half c%2). The tiny
transformer params are replicated; each core of a pair redundantly computes the
64-dim transformer for its sequence (cheaper than cross-core collectives), then
produces [1024, 16000] logits. Host reassembles [4, 1024, 32000].

Model: B=4, T=1024, C=64, H=4 heads x 16, L=4, FF=256, V=32000.

Loop order: 4-chunk groups x layers x phases, so same-table-set ScalarE
activations batch together (Sqrt for both LNs, Exp for softmax) -- ~16 ACT
table loads instead of 100+.

Per 128-token chunk i, per layer l:
  phase 1: LN (bn_stats, Sqrt+recip rstd) -> xn [128,65] with a ones column
    that folds every bias (ln_b via weight row 64, blm via Wlm row 64);
    PE transpose -> xnT [65,128]; per-head q/k at partition base 0 (row-tiled
    tile_position is broken on this terminal; col-tiling works), v in 32-wide
    head strips whose first column is the ones column of Wv (softmax denom).
  phase 2: transposed scores wT[s,t] per key-chunk j<=i; causal mask for the
    diagonal block accumulates into PSUM via an identity x (-1e9 mask) matmul;
    exp fused with PSUM evacuation on ScalarE (scale=1/8), bf16 out.
  phase 3: av accumulated over j with 4 heads col-tiled (head h at PSUM
    partitions 32h..32h+31; strict h-major groups -- start=True clears
    has_written bank-wide); den lands at row 32h; one static selector matmul
    (bsel) broadcasts den rows to whole strips, one reciprocal + one
    tensor_tensor multiply normalizes and casts to bf16; out-proj via
    zero-padded Wo kills the den/garbage rows; residual add.
  phase 4: LN2 + MLP (hT = relu(W1_pad.T @ xn2T) per 128-wide ff chunk with
    per-partition b1 bias on the Relu, y accumulated over ff chunks) +
    residual; at l=3 the chunk's lm_head is emitted inline so logits DMA
    overlaps the next group's compute: logits = xT_aug @ Wlm_aug in 500-col
    PSUM tiles, evacuation split VectorE/ScalarE, bf16 stores (halves the
    dominant HBM write; host upconverts to f32).
"""

import os

import numpy as np
import ml_dtypes

import concourse.bass as bass
import concourse.mybir as mybir
import concourse.tile as tile
from concourse import bacc
from concourse.bass import ts
from concourse.bass_utils import run_bass_kernel_spmd

# model dims
T = 1024
C = 64
H = 4
D = 16
L = 4
FF = 256
V = 32000
VH = V // 2          # per-core vocab half
NT = T // 128        # 8 token chunks
VC = 500             # vocab chunk (lm head matmul N)
NVC = VH // VC       # 32 vocab chunks per core
VG = 4               # vocab chunks per staged DMA (2000 f32 = 8KB/partition)
SCALE = 1.0 / (C ** 0.5)
NEG = -1.0e9

F32 = mybir.dt.float32
BF16 = mybir.dt.bfloat16
I32 = mybir.dt.int32

_CACHE = {}

# filled with the BassKernelResults of the last run (for test.py profiling)
LAST_RESULTS = None


def _build(apply_bo, apply_b2, parts='all'):
    nc = bacc.Bacc("TRN2", target_bir_lowering=False, debug=False)

    # ---- DRAM I/O ----
    idx_d = nc.dram_tensor("idx", [128, NT], I32, kind="ExternalInput").ap()
    temb_d = nc.dram_tensor("tok_emb", [V, C], F32, kind="ExternalInput").ap()
    pemb_d = nc.dram_tensor("pos_emb", [T, C], F32, kind="ExternalInput").ap()
    wq_d = nc.dram_tensor("wq", [L, 65, C], BF16, kind="ExternalInput").ap()
    wk_d = nc.dram_tensor("wk", [L, 65, C], BF16, kind="ExternalInput").ap()
    wv_d = nc.dram_tensor("wv", [L, 65, 128], BF16, kind="ExternalInput").ap()
    wo_d = nc.dram_tensor("wo", [L, 128, C], BF16, kind="ExternalInput").ap()
    w1_d = nc.dram_tensor("w1", [L, 65, FF], BF16, kind="ExternalInput").ap()
    w2_d = nc.dram_tensor("w2", [L, 2, 128, C], BF16, kind="ExternalInput").ap()
    b1_d = nc.dram_tensor("b1c", [128, L * 2], F32, kind="ExternalInput").ap()
    wlm_d = nc.dram_tensor("wlm", [65, VH], BF16, kind="ExternalInput").ap()
    mask_d = nc.dram_tensor("maskt", [128, 128], BF16, kind="ExternalInput").ap()
    idb_d = nc.dram_tensor("identb", [128, 128], BF16, kind="ExternalInput").ap()
    idf_d = nc.dram_tensor("identf", [128, 128], F32, kind="ExternalInput").ap()
    bsel_d = nc.dram_tensor("bsel", [128, 128], F32, kind="ExternalInput").ap()
    bo_d = b2_d = None
    if apply_bo:
        bo_d = nc.dram_tensor("bo_bc", [L, C], F32, kind="ExternalInput").ap()
    if apply_b2:
        b2_d = nc.dram_tensor("b2_bc", [L, C], F32, kind="ExternalInput").ap()
    # logits stored bf16 (halves the dominant HBM write); host upconverts
    out_d = nc.dram_tensor("logits", [T, VH], BF16, kind="ExternalOutput").ap()

    with tile.TileContext(nc) as tc:
        with (
            tc.tile_pool(name="singles", bufs=1) as singles,
            tc.tile_pool(name="sb2", bufs=3) as sb2,
            tc.tile_pool(name="sb3", bufs=3) as sb3,
            tc.tile_pool(name="sb5", bufs=5) as sb5,
            tc.tile_pool(name="ln4", bufs=6) as ln4,
            tc.tile_pool(name="stage", bufs=4) as stagep,
            tc.tile_pool(name="wps", bufs=2, space="PSUM") as wps,
            tc.tile_pool(name="smallps", bufs=2, space="PSUM") as smallps,
            tc.tile_pool(name="lgps", bufs=2, space="PSUM") as lgps,
        ):
            # ---- resident SBUF tensors ----
            wq_sb = singles.tile([65, L, C], BF16, name="wq_sb")
            wk_sb = singles.tile([65, L, C], BF16, name="wk_sb")
            wv_sb = singles.tile([65, L, 128], BF16, name="wv_sb")
            wo_sb = singles.tile([128, L, C], BF16, name="wo_sb")
            w1_sb = singles.tile([65, L, FF], BF16, name="w1_sb")
            w2_sb = singles.tile([128, L, 2, C], BF16, name="w2_sb")
            b1_sb = singles.tile([128, L * 2], F32, name="b1_sb")
            wlm_sb = singles.tile([65, VH], BF16, name="wlm_sb")
            mask_sb = singles.tile([128, 128], BF16, name="mask_sb")
            idb_sb = singles.tile([128, 128], BF16, name="idb_sb")
            idf_sb = singles.tile([128, 128], F32, name="idf_sb")
            idx_sb = singles.tile([128, NT], I32, name="idx_sb")
            eps_sb = singles.tile([128, 1], F32, name="eps_sb")
            bsel_sb = singles.tile([128, 128], F32, name="bsel_sb")
            x_sb = singles.tile([128, NT, 65], F32, name="x_sb")
            kcache = [singles.tile([16, H, NT, 128], BF16, name=f"kc{l}") for l in range(L)]
            vcache = [singles.tile([128, NT, 128], BF16, name=f"vc{l}") for l in range(L)]

            for l in range(L):
                nc.sync.dma_start(out=wq_sb[:, l, :], in_=wq_d[l])
                nc.sync.dma_start(out=wk_sb[:, l, :], in_=wk_d[l])
                nc.sync.dma_start(out=wv_sb[:, l, :], in_=wv_d[l])
                nc.sync.dma_start(out=wo_sb[:, l, :], in_=wo_d[l])
                nc.sync.dma_start(out=w1_sb[:, l, :], in_=w1_d[l])
                nc.sync.dma_start(out=w2_sb[:, l, 0, :], in_=w2_d[l, 0])
                nc.sync.dma_start(out=w2_sb[:, l, 1, :], in_=w2_d[l, 1])
            nc.sync.dma_start(out=b1_sb, in_=b1_d)
            nc.sync.dma_start(out=wlm_sb, in_=wlm_d)
            nc.sync.dma_start(out=mask_sb, in_=mask_d)
            nc.sync.dma_start(out=idb_sb, in_=idb_d)
            nc.sync.dma_start(out=idf_sb, in_=idf_d)
            nc.sync.dma_start(out=bsel_sb, in_=bsel_d)
            nc.sync.dma_start(out=idx_sb, in_=idx_d)
            nc.vector.memset(eps_sb, 1e-5)

            bo_sb = b2_sb = None
            if apply_bo:
                bo_sb = singles.tile([128, L, C], F32, name="bo_sb")
                for l in range(L):
                    row = bo_d[l: l + 1, :]
                    nc.gpsimd.dma_start(
                        out=bo_sb[:, l, :],
                        in_=bass.AP(tensor=row.tensor, offset=row.offset,
                                    ap=[[0, 128], [1, C]]),
                    )
            if apply_b2:
                b2_sb = singles.tile([128, L, C], F32, name="b2_sb")
                for l in range(L):
                    row = b2_d[l: l + 1, :]
                    nc.gpsimd.dma_start(
                        out=b2_sb[:, l, :],
                        in_=bass.AP(tensor=row.tensor, offset=row.offset,
                                    ap=[[0, 128], [1, C]]),
                    )

            # ---- embeddings: x = tok_emb[idx] + pos_emb, ones col ----
            for i in range(NT):
                nc.gpsimd.indirect_dma_start(
                    out=x_sb[:, i, 0:C],
                    out_offset=None,
                    in_=temb_d,
                    in_offset=bass.IndirectOffsetOnAxis(ap=idx_sb[:, i:i + 1], axis=0),
                )
                pos_t = sb3.tile([128, C], F32, name="pos_t")
                nc.sync.dma_start(out=pos_t, in_=pemb_d[ts(i, 128), :])
                nc.vector.tensor_add(out=x_sb[:, i, 0:C], in0=x_sb[:, i, 0:C], in1=pos_t)
            nc.vector.memset(x_sb[:, :, 64:65], 1.0)

            def layernorm(i, which, l):
                """x_sb[:,i,:64] -> normalized bf16 xn tile [128,65] (col 64 = 1).

                All-DVE: no ACT Sqrt (keeps the single Exp act-table set).
                rstd = (E[x^2] - m^2 + eps)^-0.5 via the pow ALU op.
                """
                xsq = ln4.tile([128, C], F32, name="ln_xsq")
                q = ln4.tile([128, 1], F32, name="ln_q")
                s = ln4.tile([128, 1], F32, name="ln_s")
                m = ln4.tile([128, 1], F32, name="ln_m")
                msq = ln4.tile([128, 1], F32, name="ln_msq")
                v1 = ln4.tile([128, 1], F32, name="ln_v1")
                rstd = ln4.tile([128, 1], F32, name="ln_rstd")
                x = x_sb[:, i, 0:C]
                nc.vector.tensor_tensor_reduce(
                    out=xsq, in0=x, in1=x, scale=1.0 / C, scalar=0.0,
                    op0=mybir.AluOpType.mult, op1=mybir.AluOpType.add,
                    accum_out=q)
                nc.vector.tensor_reduce(out=s, in_=x, axis=mybir.AxisListType.X,
                                        op=mybir.AluOpType.add)
                nc.vector.tensor_scalar(out=m, in0=s, scalar1=1.0 / C,
                                        scalar2=None, op0=mybir.AluOpType.mult)
                nc.vector.tensor_tensor(out=msq, in0=m, in1=m,
                                        op=mybir.AluOpType.mult)
                nc.vector.tensor_scalar(out=v1, in0=q, scalar1=msq, scalar2=1e-5,
                                        op0=mybir.AluOpType.subtract,
                                        op1=mybir.AluOpType.add)
                nc.vector.tensor_scalar(out=rstd, in0=v1, scalar1=-0.5,
                                        scalar2=None, op0=mybir.AluOpType.pow)
                xn = sb2.tile([128, 65], BF16, name=f"xn{which}")
                nc.vector.tensor_scalar(
                    out=xn[:, 0:C], in0=x,
                    scalar1=m, scalar2=rstd,
                    op0=mybir.AluOpType.subtract, op1=mybir.AluOpType.mult,
                )
                nc.vector.memset(xn[:, 64:65], 1.0)
                return xn

            def transpose65(xn, name, dtype):
                """[128,65] -> bf16 [65,128] in SBUF via PE transpose (bf16 PSUM
                out so the evacuating copy runs in the DVE 2x mode)."""
                ident = idb_sb if xn.dtype == BF16 else idf_sb
                tp = smallps.tile([65, 128], BF16, name="tp_ps", tag="smallps")
                nc.tensor.transpose(out=tp, in_=xn, identity=ident)
                xt = sb2.tile([65, 128], dtype, name=name)
                nc.vector.tensor_copy(out=xt, in_=tp)
                return xt

            lvl = {'emb': 0, 'nolm': 6, 'all': 7}.get(parts, parts)
            do_lm = lvl >= 7
            G = 4
            qt_of, pt_of = {}, {}
            for grp_base in range(0, NT, G):
                grp = range(grp_base, min(grp_base + G, NT))
                for l in range(L if lvl >= 1 else 0):
                  # phase 1: LN1 (Sqrt) + transpose + q/k/v for the whole group
                  for i in grp:
                    xn = layernorm(i, 1, l)
                    xnt = transpose65(xn, "xnt", BF16)

                    if lvl < 2:
                        continue
                    q_ps = smallps.tile([16, 512], F32, name="q_ps", tag="smallps")
                    k_ps = smallps.tile([16, 512], F32, name="k_ps", tag="smallps")
                    for h in range(H):
                        nc.tensor.matmul(out=q_ps[:, ts(h, 128)],
                                         lhsT=wq_sb[:, l, 16 * h: 16 * h + D],
                                         rhs=xnt, start=True, stop=True)
                        nc.tensor.matmul(out=k_ps[:, ts(h, 128)],
                                         lhsT=wk_sb[:, l, 16 * h: 16 * h + D],
                                         rhs=xnt, start=True, stop=True)
                    qt = sb5.tile([16, 512], BF16, name="qt")
                    qt_of[i] = qt
                    nc.vector.tensor_copy(out=qt, in_=q_ps)
                    nc.scalar.copy(out=kcache[l][:, :, i, :], in_=k_ps)

                    v_ps = smallps.tile([128, 128], F32, name="v_ps", tag="smallps")
                    nc.tensor.matmul(out=v_ps, lhsT=xnt, rhs=wv_sb[:, l, :],
                                     start=True, stop=True)
                    nc.vector.tensor_copy(out=vcache[l][:, i, :], in_=v_ps)

                  # phase 2: scores + exp (one Exp table visit per group)
                  for i in grp:
                    if lvl < 3:
                        continue
                    qt = qt_of[i]
                    pt = sb5.tile([128, (i + 1) * 512], BF16, name="pt")
                    pt_of[i] = pt
                    for g in range(0, i + 1, 2):
                        js = [j for j in (g, g + 1) if j <= i]
                        w_ps = wps.tile([128, 512 * len(js)], F32, name="w_ps")
                        for j in js:
                            off = (j - g) * 512
                            for h in range(H):
                                o = w_ps[:, off + h * 128: off + (h + 1) * 128]
                                if j == i:
                                    nc.tensor.matmul(
                                        out=o, lhsT=idb_sb, rhs=mask_sb,
                                        start=True, stop=False)
                                nc.tensor.matmul(
                                    out=o,
                                    lhsT=kcache[l][:, h, j, :],
                                    rhs=qt[:, ts(h, 128)],
                                    start=(j != i), stop=True,
                                )
                        nc.scalar.activation(
                            out=pt[:, g * 512: g * 512 + 512 * len(js)],
                            in_=w_ps,
                            func=mybir.ActivationFunctionType.Exp, scale=SCALE)

                  # phase 3: av + normalize + out-proj + residual
                  for i in grp:
                    if lvl < 4:
                        continue
                    pt = pt_of[i]
                    at_ps = smallps.tile([128, 128], F32, name="at_ps", tag="smallps")
                    # h-major: start=True clears has_written for the whole
                    # bank, so each head's accumulation group must fully close
                    # before the next head's opens (finished values survive a
                    # bit-clear; open groups do not).
                    for h in range(H):
                        for j in range(i + 1):
                            nc.tensor.matmul(
                                out=at_ps[32 * h: 32 * h + 32, :],
                                lhsT=vcache[l][:, j, 32 * h: 32 * h + 32],
                                rhs=pt[:, (4 * j + h) * 128: (4 * j + h + 1) * 128],
                                start=(j == 0), stop=(j == i),
                                tile_position=(0, 32 * h),
                            )
                    at_un = sb2.tile([128, 128], F32, name="at_un")
                    nc.vector.tensor_copy(out=at_un, in_=at_ps)
                    # broadcast each head's den row (at partition 32h) to its whole
                    # 32-row strip with one selector matmul (bsel[k,p]=1 iff
                    # k==32*(p//32)), then one full-width reciprocal
                    den_ps = smallps.tile([128, 128], F32, name="den_ps",
                                          tag="smallps")
                    nc.tensor.matmul(out=den_ps, lhsT=bsel_sb, rhs=at_un,
                                     start=True, stop=True)
                    at_n = sb2.tile([128, 128], BF16, name="at_n")
                    nc.vector.tensor_tensor(out=at_n, in0=at_un, in1=den_ps,
                                            op=mybir.AluOpType.divide)

                    if lvl < 5:
                        continue
                    xo_ps = smallps.tile([128, C], F32, name="xo_ps", tag="smallps")
                    nc.tensor.matmul(out=xo_ps, lhsT=at_n, rhs=wo_sb[:, l, :],
                                     start=True, stop=True)
                    nc.vector.tensor_add(out=x_sb[:, i, 0:C], in0=x_sb[:, i, 0:C],
                                         in1=xo_ps)
                    if apply_bo:
                        nc.vector.tensor_add(out=x_sb[:, i, 0:C],
                                             in0=x_sb[:, i, 0:C], in1=bo_sb[:, l, :])

                  # phase 4: LN2 (Sqrt) + MLP for the whole group
                  for i in grp:
                    if lvl < 6:
                        continue
                    xn2 = layernorm(i, 2, l)
                    xn2t = transpose65(xn2, "xn2t", BF16)
                    y_ps = smallps.tile([128, C], F32, name="y_ps", tag="smallps")
                    h_ps = smallps.tile([128, 2, 128], F32, name="h_ps",
                                        tag="smallps")
                    for n in range(2):
                        nc.tensor.matmul(out=h_ps[:, n, :],
                                         lhsT=w1_sb[:, l, ts(n, 128)],
                                         rhs=xn2t, start=True, stop=True)
                    h_sb = sb2.tile([128, 2, 128], BF16, name="h_sb")
                    # relu(x + b1) on DVE (add then max-with-0) keeps ACT free
                    # for exp/evac and off the Relu act table
                    for n in range(2):
                        nc.vector.tensor_scalar(
                            out=h_sb[:, n, :], in0=h_ps[:, n, :],
                            scalar1=b1_sb[:, 2 * l + n: 2 * l + n + 1],
                            scalar2=0.0,
                            op0=mybir.AluOpType.add, op1=mybir.AluOpType.max)
                    for n in range(2):
                        nc.tensor.matmul(out=y_ps, lhsT=h_sb[:, n, :],
                                         rhs=w2_sb[:, l, n, :],
                                         start=(n == 0), stop=(n == 1))
                    nc.vector.tensor_add(out=x_sb[:, i, 0:C], in0=x_sb[:, i, 0:C],
                                         in1=y_ps)
                    if apply_b2:
                        nc.vector.tensor_add(out=x_sb[:, i, 0:C],
                                             in0=x_sb[:, i, 0:C], in1=b2_sb[:, l, :])

                    # ---- lm head, emitted right after this chunk's last layer
                    if not (do_lm and l == L - 1):
                        continue
                    xt = transpose65(x_sb[:, i, :], "xt_lm", BF16)
                    for kg in range(NVC // VG):
                        st = stagep.tile([128, VG * VC], BF16, name="lg_stage")
                        for u in range(VG):
                            k = kg * VG + u
                            lg_ps = lgps.tile([128, VC], F32, name="lg_ps")
                            nc.tensor.matmul(out=lg_ps, lhsT=xt,
                                             rhs=wlm_sb[:, k * VC:(k + 1) * VC],
                                             start=True, stop=True)
                            dst = st[:, u * VC:(u + 1) * VC]
                            if k % 8 < 3:
                                nc.vector.tensor_copy(out=dst, in_=lg_ps)
                            else:
                                nc.scalar.copy(out=dst, in_=lg_ps)
                        nc.sync.dma_start(
                            out=out_d[ts(i, 128), kg * VG * VC:(kg + 1) * VG * VC],
                            in_=st)
            if not do_lm:
                xdbg_d = nc.dram_tensor("xdbg", [128, NT * 65], F32,
                                        kind="ExternalOutput").ap()
                nc.sync.dma_start(out=xdbg_d,
                                  in_=x_sb.rearrange("p a b -> p (a b)"))
    nc.compile()
    return nc


def _prep_inputs(idx, tok_emb, pos_emb, Wq, Wk, Wv, Wo, bo, W1, b1, W2, b2,
                 ln1_g, ln1_b, ln2_g, ln2_b, Wlm, blm):
    """Host-side weight layout prep. Returns (common dict, per-core dicts, flags)."""
    f32 = np.float32
    bf16 = ml_dtypes.bfloat16
    Wq, Wk, Wv, Wo = f32(Wq), f32(Wk), f32(Wv), f32(Wo)
    W1, W2, Wlm = f32(W1), f32(W2), f32(Wlm)
    ln1_g, ln1_b, ln2_g, ln2_b = f32(ln1_g), f32(ln1_b), f32(ln2_g), f32(ln2_b)
    bo, b1, b2, blm = f32(bo), f32(b1), f32(b2), f32(blm)

    wq_np = np.zeros((L, 65, C), f32)
    wk_np = np.zeros((L, 65, C), f32)
    wv_np = np.zeros((L, 65, 128), f32)
    wo_np = np.zeros((L, 128, C), f32)
    w1_np = np.zeros((L, 65, FF), f32)
    for l in range(L):
        g1, b1n = ln1_g[l], ln1_b[l]
        g2, b2n = ln2_g[l], ln2_b[l]
        for h in range(H):
            wq_np[l, 0:C, 16 * h:16 * h + D] = g1[:, None] * Wq[l, h]
            wq_np[l, 64, 16 * h:16 * h + D] = b1n @ Wq[l, h]
            wk_np[l, 0:C, 16 * h:16 * h + D] = g1[:, None] * Wk[l, h]
            wk_np[l, 64, 16 * h:16 * h + D] = b1n @ Wk[l, h]
            wv_np[l, 0:C, 32 * h + 1:32 * h + 1 + D] = g1[:, None] * Wv[l, h]
            wv_np[l, 64, 32 * h + 1:32 * h + 1 + D] = b1n @ Wv[l, h]
            wv_np[l, 64, 32 * h] = 1.0
            wo_np[l, 32 * h + 1:32 * h + 1 + D, :] = Wo[l, 16 * h:16 * h + D, :]
        w1_np[l, 0:C, :] = g2[:, None] * W1[l]
        w1_np[l, 64, :] = b2n @ W1[l]
    w2_np = W2.reshape(L, 2, 128, C)
    b1c_np = np.ascontiguousarray(
        b1.reshape(L * 2, 128).T)  # [128, L*2]

    sidx = np.arange(128)
    mask_np = np.where(sidx[:, None] <= sidx[None, :], 0.0, NEG).astype(f32)
    ident_np = np.eye(128, dtype=f32)
    bsel_np = np.zeros((128, 128), f32)
    bsel_np[32 * (sidx // 32), sidx] = 1.0

    common = {
        "tok_emb": np.ascontiguousarray(tok_emb, f32),
        "pos_emb": np.ascontiguousarray(pos_emb, f32),
        "wq": wq_np.astype(bf16), "wk": wk_np.astype(bf16),
        "wv": wv_np.astype(bf16), "wo": wo_np.astype(bf16),
        "w1": w1_np.astype(bf16), "w2": w2_np.astype(bf16),
        "b1c": b1c_np,
        "maskt": mask_np.astype(bf16),
        "identb": ident_np.astype(bf16),
        "identf": ident_np,
        "bsel": bsel_np,
    }
    apply_bo = bool(np.any(bo != 0))
    apply_b2 = bool(np.any(b2 != 0))
    if apply_bo:
        common["bo_bc"] = np.ascontiguousarray(bo, f32)
    if apply_b2:
        common["b2_bc"] = np.ascontiguousarray(b2, f32)

    wlm_aug = np.concatenate([Wlm, blm[None, :]], axis=0)  # [65, V]
    idx_i = np.asarray(idx).astype(np.int32)

    per_core = []
    for c in range(8):
        b, half = c // 2, c % 2
        m = dict(common)
        m["idx"] = np.ascontiguousarray(idx_i[b].reshape(NT, 128).T)  # [128, NT]
        m["wlm"] = np.ascontiguousarray(wlm_aug[:, half * VH:(half + 1) * VH]).astype(bf16)
        per_core.append(m)
    return per_core, apply_bo, apply_b2


def kernel(**inputs):
    global LAST_RESULTS
    per_core, apply_bo, apply_b2 = _prep_inputs(**inputs)

    key = (apply_bo, apply_b2)
    if key not in _CACHE:
        _CACHE[key] = _build(apply_bo, apply_b2)
    nc = _CACHE[key]

    trace = os.environ.get("KERNEL_TRACE", "0") == "1"
    if trace:
        try:
            from antenv.axon_hooks import get_axon_ntff_profile_hook  # noqa: F401
        except ImportError:
            trace = False  # no NTFF path in this container
    res = run_bass_kernel_spmd(nc, per_core, core_ids=list(range(8)), trace=trace)
    LAST_RESULTS = res

    out = np.empty((4, T, V), np.float32)
    for c in range(8):
        b, half = c // 2, c % 2
        out[b, :, half * VH:(half + 1) * VH] = np.float32(res.results[c]["logits"])
    return out

